# revision 1
# baseline (speedup 1.0000x reference)
"""Trainium2 Bass kernel for per-sequence-index attention with MLP projections.

Reference computation (per batch row b, sequence index s):
  q/k/v = relu(LayerNorm(x @ W + b; gamma, beta))      [B,S,64] each
  att[:, s] = (q_s @ k_s^T) @ v_s                      (no softmax)

Strategy:
  - Shard S across 8 cores (embarrassingly parallel), S_local = 256.
  - Tokens ordered (s, b) s-major so each s-step's 256 tokens are contiguous.
  - Host pre-transposes x to xT[f, s*B+b] (bf16) so projections run with the
    small weight matrices as the PE-stationary operand.
  - LayerNorm mean is folded into the weights (columns centered); gamma is
    folded into W; variance comes from an ACT square pass + a PE ones-matmul
    with 1/gamma^2/64 weights; rstd = reciprocal_approx_fast(sqrt(var+eps)).
  - rstd rows are partition-broadcast by GPSIMD; DVE applies the scale while
    evacuating PSUM; ACT applies relu(z + beta) and casts to bf16.
  - k, v are DMA-transposed (bf16 xbar) to token-major tiles.
  - Attention uses associativity: att_s = q_s @ (k_s^T @ v_s), so the only
    per-s intermediate is a 64x64 matrix.
"""

import os

import numpy as np
import ml_dtypes

import concourse.bass as bass
import concourse.mybir as mybir
import concourse.tile as tile
from concourse.bass_utils import run_bass_kernel_spmd

BF16 = ml_dtypes.bfloat16
B, S, F, D = 256, 2048, 256, 64
NCORES = 8
SL = S // NCORES            # 256 s-steps per core
EPS = 1e-5
FP32 = mybir.dt.float32
BF16_DT = mybir.dt.bfloat16


def _legalize_waits(nc):
    """Split multi-wait instructions into chained single-wait NoOps.

    The TRN2 instruction encoding has one sync-wait slot; this walrus build
    refuses to split waits itself ("Too many sync wait commands").
    """
    nsplit = 0
    for f in nc.m.functions:
        for blk in f.blocks:
            out = []
            changed = False
            for i in blk.instructions:
                si = getattr(i, 'sync_info', None)
                if si is not None and si.on_wait and len(si.on_wait) > 1:
                    waits = list(si.on_wait)
                    for w in waits[:-1]:
                        out.append(mybir.InstNoOp(
                            name=f"{i.name}_wsplit{nsplit}",
                            ins=[], outs=[],
                            sync_info=mybir.SyncInfo(on_wait=[w], on_update=[]),
                            engine=i.engine))
                        nsplit += 1
                    i.sync_info = mybir.SyncInfo(
                        on_wait=[waits[-1]], on_update=list(si.on_update or []))
                    changed = True
                out.append(i)
            if changed:
                blk.instructions = out
    return nsplit


def _act_rsqrt(nc, out, in_, bias_ap):
    # Raw InstActivation with func=Rsqrt (bass-level wrapper bans it; the
    # hardware LUT measures ~4e-5 rel err, plenty for this kernel).
    eng = nc.scalar
    ins = [eng.lower_ap(in_), eng.lower_ap(bias_ap),
           mybir.ImmediateValue(dtype=mybir.dt.float32, value=1.0),
           mybir.ImmediateValue(dtype=mybir.dt.float32, value=0.0)]
    return eng.add_instruction(mybir.InstActivation(
        name=nc.get_next_instruction_name(),
        func=mybir.ActivationFunctionType.Rsqrt,
        ins=ins, outs=[eng.lower_ap(out)]))


def build_kernel(T):
    """Build the Bass program for one core processing T tokens (T % 1024 == 0)."""
    nblk = T // 512          # 512-token blocks (= 2 s-steps each)
    npair = nblk // 2

    nc = bass.Bass("TRN2")
    xT = nc.dram_tensor("xT", [2, 128, T], BF16_DT, kind="ExternalInput")
    wqkv = nc.dram_tensor("wqkv", [2, 128, 192], BF16_DT, kind="ExternalInput")
    bqkv = nc.dram_tensor("bqkv", [1, 192], BF16_DT, kind="ExternalInput")
    onesb_qk = nc.dram_tensor("onesb_qk", [128, 128], FP32, kind="ExternalInput")
    onesb_v = nc.dram_tensor("onesb_v", [128, 128], FP32, kind="ExternalInput")
    betas = nc.dram_tensor("betas", [128, 2], FP32, kind="ExternalInput")
    out = nc.dram_tensor("att", [B, T // B, D], FP32, kind="ExternalOutput")

    with tile.TileContext(nc) as tc:
        with (
            tc.tile_pool(name="singles", bufs=1) as singles,
            tc.tile_pool(name="xc", bufs=4) as xc_pool,
            tc.tile_pool(name="sq", bufs=2) as sq_pool,
            tc.tile_pool(name="sr", bufs=2) as sr_pool,
            tc.tile_pool(name="rb", bufs=2) as rb_pool,
            tc.tile_pool(name="zq", bufs=2) as z_pool,
            tc.tile_pool(name="qn", bufs=2) as qn_pool,
            tc.tile_pool(name="tm", bufs=5) as tm_pool,
            tc.tile_pool(name="msb", bufs=2) as msb_pool,
            tc.tile_pool(name="ao", bufs=2) as ao_pool,
            tc.tile_pool(name="qkps", bufs=2, space="PSUM") as qkps_pool,
            tc.tile_pool(name="vps", bufs=1, space="PSUM") as vps_pool,
            tc.tile_pool(name="stps", bufs=1, space="PSUM") as stps_pool,
            tc.tile_pool(name="mps", bufs=1, space="PSUM") as mps_pool,
            tc.tile_pool(name="probe", bufs=1, space="PSUM") as probe_pool,
            tc.tile_pool(name="atps", bufs=1, space="PSUM") as atps_pool,
        ):
            # ---- constants ----
            w_sb = [singles.tile([128, 192], BF16_DT, name=f"w{c}", tag=f"w{c}")
                    for c in (0, 1)]
            for c in (0, 1):
                nc.sync.dma_start(out=w_sb[c], in_=wqkv[c])
            b_sb = singles.tile([1, 192], BF16_DT)
            nc.sync.dma_start(out=b_sb, in_=bqkv[:])
            ob_qk = singles.tile([128, 128], FP32)
            nc.sync.dma_start(out=ob_qk, in_=onesb_qk[:])
            ob_v = singles.tile([128, 128], FP32)
            nc.sync.dma_start(out=ob_v, in_=onesb_v[:])
            beta_sb = singles.tile([128, 2], FP32)
            nc.sync.dma_start(out=beta_sb, in_=betas[:])
            ones_row = singles.tile([1, 512], BF16_DT)
            nc.vector.memset(ones_row, 1.0)
            eps_sb = singles.tile([128, 1], FP32)
            nc.vector.memset(eps_sb, EPS)

            # Warm up constant tiles on PE so their DMA-completion waits are
            # absorbed once here (matmul instructions can carry only one sync
            # wait on hardware). wrm also serves as the scratch target for the
            # per-pair dependency-absorbing probe matmuls.
            wrm = probe_pool.tile([1, 128], FP32)
            probe_col = [0]
            for ci, cst in enumerate((w_sb[0][:, 0:1], w_sb[1][:, 0:1],
                                      ob_qk[:, 0:1], ob_v[:, 0:1],
                                      b_sb[0:1, 0:1], ones_row[0:1, 0:1])):
                nc.tensor.matmul(out=wrm[0:1, ci:ci + 1], lhsT=cst,
                                 rhs=cst, start=True, stop=True)
            probe_col[0] = 6

            # Warm the memset constants into ACT's vector clock so Sqrt/Relu
            # bias operands never add a second sync wait.
            warm_act = singles.tile([128, 2], FP32)
            nc.scalar.copy(out=warm_act[:, 0:1], in_=eps_sb)
            nc.scalar.copy(out=warm_act[:, 1:2], in_=beta_sb[:, 1:2])

            def probe(ap):
                # Tiny PE matmul reading `ap` so a following matmul does not
                # need its own sync wait for that producer (1-wait HW limit).
                c = probe_col[0] % 128
                probe_col[0] += 1
                nc.tensor.matmul(out=wrm[0:1, c:c + 1], lhsT=ap[:, 0:1],
                                 rhs=ap[:, 0:1], start=True, stop=True)
            prev_z = []

            for p in range(npair):
                qkps = [None, None]
                qkn = [None, None]
                # absorb previous pair's DVE ticks into PE's clock so this
                # pair's projection matmuls carry only their xc DMA wait
                for zp in prev_z:
                    probe(zp)
                prev_z = []
                vps = vps_pool.tile([128, 512], FP32)

                # ---------- projections + squares + stats ----------
                for h in (0, 1):
                    t0 = p * 1024 + h * 512
                    xc0 = xc_pool.tile([128, 512], BF16_DT, tag="xc0")
                    xc1 = xc_pool.tile([128, 512], BF16_DT, tag="xc1")
                    nc.sync.dma_start(out=xc0, in_=xT[0, :, t0:t0 + 512])
                    nc.sync.dma_start(out=xc1, in_=xT[1, :, t0:t0 + 512])

                    ps = qkps_pool.tile([128, 512], FP32, name="ps")
                    qkps[h] = ps
                    # q^T on partitions 0:64, k^T on 64:128 (col-tiled)
                    nc.tensor.matmul(out=ps[0:64, :], lhsT=w_sb[0][:, 0:64],
                                     rhs=xc0, start=True, stop=False)
                    nc.tensor.matmul(out=ps[64:128, :], lhsT=w_sb[0][:, 64:128],
                                     rhs=xc0, start=True, stop=False)
                    nc.tensor.matmul(out=ps[0:64, :], lhsT=w_sb[1][:, 0:64],
                                     rhs=xc1, start=False, stop=False)
                    nc.tensor.matmul(out=ps[64:128, :], lhsT=w_sb[1][:, 64:128],
                                     rhs=xc1, start=False, stop=False)
                    nc.tensor.matmul(out=ps[0:64, :], lhsT=b_sb[0:1, 0:64],
                                     rhs=ones_row, start=False, stop=True)
                    nc.tensor.matmul(out=ps[64:128, :], lhsT=b_sb[0:1, 64:128],
                                     rhs=ones_row, start=False, stop=True)

                    # v^T pair-packed: even block -> partitions 0:64, odd -> 64:128
                    nc.tensor.matmul(out=vps[64 * h:64 * h + 64, :],
                                     lhsT=w_sb[0][:, 128:192], rhs=xc0,
                                     start=True, stop=False)
                    nc.tensor.matmul(out=vps[64 * h:64 * h + 64, :],
                                     lhsT=w_sb[1][:, 128:192], rhs=xc1,
                                     start=False, stop=False)
                    nc.tensor.matmul(out=vps[64 * h:64 * h + 64, :],
                                     lhsT=b_sb[0:1, 128:192], rhs=ones_row,
                                     start=False, stop=True)

                    # squares (ACT) then weighted column-sum on PE; the fat
                    # lhsT replicates var_q to partitions 0:64, var_k to 64:128
                    sq = sq_pool.tile([128, 512], FP32, tag="sqqk")
                    nc.scalar.square(out=sq, in_=ps)
                    stqk = stps_pool.tile([128, 512], FP32, name="stqk",
                                          tag="stqk")
                    nc.tensor.matmul(out=stqk, lhsT=ob_qk, rhs=sq,
                                     start=True, stop=True)
                    rbqk = rb_pool.tile([128, 512], FP32, name="rbqk",
                                        tag="rbqk")
                    _act_rsqrt(nc, rbqk, stqk, eps_sb)
                    z = z_pool.tile([128, 512], BF16_DT, name="z", tag="zqk")
                    nc.vector.tensor_mul(out=z, in0=ps, in1=rbqk)
                    prev_z.append(z)
                    qkn[h] = qn_pool.tile([128, 512], BF16_DT, name="qkn",
                                          tag="qkn")
                    nc.scalar.activation(out=qkn[h], in_=z,
                                         func=mybir.ActivationFunctionType.Relu,
                                         bias=beta_sb[:, 0:1], scale=1.0)

                # ---------- v-pair stats + normalize ----------
                sq_v = sq_pool.tile([128, 512], FP32, tag="sqv")
                nc.scalar.square(out=sq_v, in_=vps)
                stv = stps_pool.tile([128, 512], FP32, name="stv", tag="stv")
                nc.tensor.matmul(out=stv, lhsT=ob_v, rhs=sq_v,
                                 start=True, stop=True)
                rb_v = rb_pool.tile([128, 512], FP32, tag="rbv")
                _act_rsqrt(nc, rb_v, stv, eps_sb)
                z_v = z_pool.tile([128, 512], BF16_DT, tag="zv")
                nc.vector.tensor_mul(out=z_v, in0=vps, in1=rb_v)
                prev_z.append(z_v)
                vn = qn_pool.tile([128, 512], BF16_DT, tag="vn")
                nc.scalar.activation(out=vn, in_=z_v,
                                     func=mybir.ActivationFunctionType.Relu,
                                     bias=beta_sb[:, 1:2], scale=1.0)

                # ---------- transposes to token-major (bf16 xbar DMA) ----------
                # qk_tm[h][c][:, 64:128] = k token-major for block-local tokens
                # v_tm[c][:, 64h:64h+64] = v token-major (h = even/odd block)
                qk_tm = [[None] * 4, [None] * 4]
                v_tm = [None] * 4
                for c in range(4):
                    v_tm[c] = tm_pool.tile([128, 128], BF16_DT, name="vtm", tag="vtm")
                    nc.scalar.dma_start_transpose(
                        out=v_tm[c], in_=vn[:, 128 * c:128 * c + 128])
                    probe(v_tm[c])
                    for h in (0, 1):
                        qk_tm[h][c] = tm_pool.tile([128, 128], BF16_DT, name=f"qktm{h}", tag=f"qktm{h}")
                        nc.scalar.dma_start_transpose(
                            out=qk_tm[h][c],
                            in_=qkn[h][:, 128 * c:128 * c + 128])

                # ---------- attention ----------
                for h in (0, 1):
                    i_blk = 2 * p + h
                    mps = mps_pool.tile([64, 128], FP32)
                    for ss in (0, 1):
                        for cc in (0, 1):
                            chunk = 2 * ss + cc
                            nc.tensor.matmul(
                                out=mps[:, 64 * ss:64 * ss + 64],
                                lhsT=qk_tm[h][chunk][:, 64:128],
                                rhs=v_tm[chunk][:, 64 * h:64 * h + 64],
                                start=(cc == 0), stop=(cc == 1))
                    msb = msb_pool.tile([64, 128], BF16_DT)
                    nc.scalar.copy(out=msb, in_=mps)

                    atps = atps_pool.tile([128, 256], FP32)
                    for bt in (0, 1):
                        for ss in (0, 1):
                            nc.tensor.matmul(
                                out=atps[:, 128 * bt + 64 * ss:128 * bt + 64 * ss + 64],
                                lhsT=qkn[h][0:64, 256 * ss + 128 * bt:256 * ss + 128 * bt + 128],
                                rhs=msb[:, 64 * ss:64 * ss + 64],
                                start=True, stop=True)
                    atts = ao_pool.tile([128, 256], FP32)
                    nc.scalar.copy(out=atts, in_=atps)
                    for bt in (0, 1):
                        nc.sync.dma_start(
                            out=out[128 * bt:128 * bt + 128,
                                    2 * i_blk:2 * i_blk + 2, :],
                            in_=atts[:, 128 * bt:128 * bt + 128].rearrange(
                                "p (s d) -> p s d", s=2))
    _legalize_waits(nc)
    return nc


def prepare_host_inputs(inputs):
    """Precompute the shared (weight-derived) device inputs."""
    def fold(W, b, g):
        Wc = W - W.mean(axis=1, keepdims=True)
        bc = b - b.mean()
        return (Wc * g[None, :]), (bc * g)

    Wq, bq = fold(inputs["Wq"], inputs["bq"], inputs["gq"])
    Wk, bk = fold(inputs["Wk"], inputs["bk"], inputs["gk"])
    Wv, bv = fold(inputs["Wv"], inputs["bv"], inputs["gv"])

    wqkv = np.concatenate([Wq, Wk, Wv], axis=1).reshape(2, 128, 192)
    bqkv = np.concatenate([bq, bk, bv]).reshape(1, 192)

    inv2q = (1.0 / np.square(inputs["gq"])) / D
    inv2k = (1.0 / np.square(inputs["gk"])) / D
    inv2v = (1.0 / np.square(inputs["gv"])) / D
    onesb_qk = np.zeros((128, 128), np.float32)
    onesb_qk[0:64, 0:64] = inv2q[:, None]
    onesb_qk[64:128, 64:128] = inv2k[:, None]
    onesb_v = np.zeros((128, 128), np.float32)
    onesb_v[0:64, 0:64] = inv2v[:, None]
    onesb_v[64:128, 64:128] = inv2v[:, None]
    betas = np.zeros((128, 2), np.float32)
    betas[0:64, 0] = inputs["betaq"]
    betas[64:128, 0] = inputs["betak"]
    betas[0:64, 1] = inputs["betav"]
    betas[64:128, 1] = inputs["betav"]

    return {
        "wqkv": np.ascontiguousarray(wqkv.astype(BF16)),
        "bqkv": np.ascontiguousarray(bqkv.astype(BF16)),
        "onesb_qk": onesb_qk,
        "onesb_v": onesb_v,
        "betas": betas,
    }


def make_xT(x_bf16, s0, s1):
    """x[b, s0:s1, f] -> xT[chunk, p, s_local*B + b] (bf16)."""
    xc = x_bf16[:, s0:s1, :]                      # [B, SLc, F]
    xt = np.transpose(xc, (2, 1, 0))              # [F, SLc, B]
    return np.ascontiguousarray(xt.reshape(2, 128, -1))


_cached = {}
_last_results = {}


def kernel(**inputs):
    x = np.asarray(inputs["inputs"], np.float32)
    Bx, Sx, Fx = x.shape
    assert (Bx, Sx, Fx) == (B, S, F)

    T = SL * B
    if "nc" not in _cached:
        _cached["nc"] = build_kernel(T)
    nc = _cached["nc"]

    shared = prepare_host_inputs({k: np.asarray(v, np.float32)
                                  for k, v in inputs.items() if k != "inputs"})
    x_bf16 = x.astype(BF16)
    in_maps = []
    for core in range(NCORES):
        m = dict(shared)
        m["xT"] = make_xT(x_bf16, core * SL, (core + 1) * SL)
        in_maps.append(m)

    trace = bool(os.environ.get("BASS_TRACE"))
    res = run_bass_kernel_spmd(nc, in_maps, core_ids=list(range(NCORES)),
                               trace=trace)
    _last_results["res"] = res
    # per-core output is [B, SL, D]; concatenate along s
    return np.concatenate([r["att"] for r in res.results], axis=1)


if __name__ == "__main__":
    rng = np.random.default_rng(0)
    fake = {
        "inputs": rng.standard_normal((B, S, F), dtype=np.float32),
        "Wq": rng.standard_normal((F, D), dtype=np.float32) / 16,
        "bq": rng.standard_normal(D).astype(np.float32) * 0.01,
        "gq": 1 + rng.standard_normal(D).astype(np.float32) * 0.01,
        "Wk": rng.standard_normal((F, D), dtype=np.float32) / 16,
        "bk": rng.standard_normal(D).astype(np.float32) * 0.01,
        "gk": 1 + rng.standard_normal(D).astype(np.float32) * 0.01,
        "Wv": rng.standard_normal((F, D), dtype=np.float32) / 16,
        "bv": rng.standard_normal(D).astype(np.float32) * 0.01,
        "gv": 1 + rng.standard_normal(D).astype(np.float32) * 0.01,
        "betaq": rng.standard_normal(D).astype(np.float32) * 0.01,
        "betak": rng.standard_normal(D).astype(np.float32) * 0.01,
        "betav": rng.standard_normal(D).astype(np.float32) * 0.01,
    }
    out = kernel(**fake)
    print("kernel output", out.shape, out.dtype, float(np.abs(out).max()))



# revision 14
# speedup vs baseline: 2.6057x; 2.6057x over previous
"""Trainium2 Bass kernel for per-sequence-index attention with MLP projections.

Reference computation (per batch row b, sequence index s):
  q/k/v = relu(LayerNorm(x @ W + b; gamma, beta))      [B,S,64] each
  att[:, s] = (q_s @ k_s^T) @ v_s                      (no softmax)

Strategy (v2 — rebalanced across engines):
  - Shard S across 8 cores (embarrassingly parallel), S_local = 256.
  - Tokens ordered (s, b) s-major; host pre-transposes x to xT[f, tok] bf16.
  - LayerNorm mean folded into centered weights, gamma folded into W; the
    variance comes from a bf16 ACT-square pass + a small bf16 PE matmul with
    1/(64 g^2) weights that deposits compact per-token variance rows; one
    raw-Rsqrt ACT pass per 1024 tokens covers q/k/v at once.
  - GPSIMD partition-broadcast replicates the compact rstd rows to full
    tiles; DVE applies the scale (tensor_mul) and beta+relu (tensor_scalar
    add/max) — keeping the ScalarE budget small.
  - q|k packed in one 128-row matmul per F-chunk; the h=1 block uses a
    swapped (k top / q bottom) layout so the attention matmuls of the two
    blocks land on disjoint PE row strips and run concurrently.
  - k and v are DMA-transposed to token-major with ONE batched xbar
    transpose each ([64,512] -> [128,4,64]).
  - Attention uses associativity: att_s = q_s @ (k_s^T @ v_s); the four
    64x64 kTv matrices of a pair live col-packed in one PSUM bank.
"""

import os

import numpy as np
import ml_dtypes

import concourse.bass as bass
import concourse.mybir as mybir
import concourse.tile as tile
from concourse.bass_utils import run_bass_kernel_spmd

BF16 = ml_dtypes.bfloat16
B, S, F, D = 256, 2048, 256, 64
NCORES = 8
SL = S // NCORES            # 256 s-steps per core
EPS = 1e-5
FP32 = mybir.dt.float32
BF16_DT = mybir.dt.bfloat16


def _legalize_waits(nc):
    """Split multi-wait instructions into chained single-wait NoOps.

    The TRN2 instruction encoding has one sync-wait slot; this walrus build
    refuses to split waits itself ("Too many sync wait commands").
    """
    nsplit = 0
    for f in nc.m.functions:
        for blk in f.blocks:
            out = []
            changed = False
            for i in blk.instructions:
                si = getattr(i, 'sync_info', None)
                if si is not None and si.on_wait and len(si.on_wait) > 1:
                    waits = list(si.on_wait)
                    for w in waits[:-1]:
                        out.append(mybir.InstNoOp(
                            name=f"{i.name}_wsplit{nsplit}",
                            ins=[], outs=[],
                            sync_info=mybir.SyncInfo(on_wait=[w], on_update=[]),
                            engine=i.engine))
                        nsplit += 1
                    i.sync_info = mybir.SyncInfo(
                        on_wait=[waits[-1]], on_update=list(si.on_update or []))
                    changed = True
                out.append(i)
            if changed:
                blk.instructions = out
    return nsplit


def _act_rsqrt(nc, out, in_, bias_ap):
    # Raw InstActivation with func=Rsqrt (bass-level wrapper bans it; the
    # hardware LUT measures ~4e-5 rel err, plenty for this kernel).
    eng = nc.scalar
    ins = [eng.lower_ap(in_), eng.lower_ap(bias_ap),
           mybir.ImmediateValue(dtype=mybir.dt.float32, value=1.0),
           mybir.ImmediateValue(dtype=mybir.dt.float32, value=0.0)]
    return eng.add_instruction(mybir.InstActivation(
        name=nc.get_next_instruction_name(),
        func=mybir.ActivationFunctionType.Rsqrt,
        ins=ins, outs=[eng.lower_ap(out)]))


def build_kernel(T):
    """Build the Bass program for one core processing T tokens (T % 1024 == 0)."""
    npair = T // 1024

    nc = bass.Bass("TRN2")
    xT = nc.dram_tensor("xT", [2, 128, T], BF16_DT, kind="ExternalInput")
    # Packed constants (see prepare_host_inputs for layouts).
    w1d = nc.dram_tensor("w1d", [2, 128, 128], BF16_DT, kind="ExternalInput")
    b1d = nc.dram_tensor("b1d", [1, 128], BF16_DT, kind="ExternalInput")
    w2d = nc.dram_tensor("w2d", [2, 128, 64], BF16_DT, kind="ExternalInput")
    b2d = nc.dram_tensor("b2d", [1, 64], BF16_DT, kind="ExternalInput")
    obfd = nc.dram_tensor("obfd", [2, 128, 128], BF16_DT, kind="ExternalInput")
    betad = nc.dram_tensor("betad", [128, 2], FP32, kind="ExternalInput")
    out = nc.dram_tensor("att", [B, T // B, D], FP32, kind="ExternalOutput")

    with tile.TileContext(nc) as tc:
        with (
            tc.tile_pool(name="singles", bufs=1) as singles,
            tc.tile_pool(name="xc", bufs=2) as xc_pool,
            tc.tile_pool(name="sq", bufs=2) as sq_pool,
            tc.tile_pool(name="rstd", bufs=2) as rstd_pool,
            tc.tile_pool(name="zz", bufs=2) as z_pool,
            tc.tile_pool(name="qn", bufs=2) as qn_pool,
            tc.tile_pool(name="tm", bufs=2) as tm_pool,
            tc.tile_pool(name="msb", bufs=2) as msb_pool,
            tc.tile_pool(name="ao", bufs=2) as ao_pool,
            tc.tile_pool(name="qkps", bufs=2, space="PSUM") as qkps_pool,
            tc.tile_pool(name="vps", bufs=1, space="PSUM") as vps_pool,
            tc.tile_pool(name="varps", bufs=1, space="PSUM") as var_pool,
            tc.tile_pool(name="mps", bufs=1, space="PSUM") as mps_pool,
            tc.tile_pool(name="atps", bufs=1, space="PSUM") as atps_pool,
        ):
            # ---- constants ----
            w1 = [singles.tile([128, 128], BF16_DT, name=f"w1_{c}")
                  for c in (0, 1)]
            for c in (0, 1):
                nc.sync.dma_start(out=w1[c], in_=w1d[c])
            b1 = singles.tile([1, 128], BF16_DT, name="b1")
            nc.sync.dma_start(out=b1, in_=b1d[:])
            w2 = [singles.tile([128, 64], BF16_DT, name=f"w2_{c}") for c in (0, 1)]
            for c in (0, 1):
                nc.sync.dma_start(out=w2[c], in_=w2d[c])
            b2 = singles.tile([1, 64], BF16_DT)
            nc.sync.dma_start(out=b2, in_=b2d[:])
            obf = [singles.tile([128, 128], BF16_DT, name=f"obf_{g}") for g in range(2)]
            for g in range(2):
                nc.sync.dma_start(out=obf[g], in_=obfd[g])
            betas = singles.tile([128, 2], FP32)
            nc.sync.dma_start(out=betas, in_=betad[:])
            ones_row = singles.tile([1, 512], BF16_DT)
            nc.vector.memset(ones_row, 1.0)
            eps_sb = singles.tile([128, 1], FP32)
            nc.vector.memset(eps_sb, EPS)

            for p in range(npair):
                t0 = 1024 * p
                # ---------- input ----------
                xc = [xc_pool.tile([128, 1024], BF16_DT, name=f"xc{c}", tag=f"xc{c}")
                      for c in (0, 1)]
                for c in (0, 1):
                    nc.sync.dma_start(out=xc[c], in_=xT[c, :, t0:t0 + 1024])

                # ---------- projections ----------
                # ps_qk[h]: q rows 0:64 | k rows 64:128
                ps_qk = []
                for h in (0, 1):
                    ps = qkps_pool.tile([128, 512], FP32, name=f"psqk{h}", tag="psqk")
                    ps_qk.append(ps)
                    nc.tensor.matmul(out=ps, lhsT=w1[0],
                                     rhs=xc[0][:, 512 * h:512 * h + 512],
                                     start=True, stop=False)
                    nc.tensor.matmul(out=ps, lhsT=w1[1],
                                     rhs=xc[1][:, 512 * h:512 * h + 512],
                                     start=False, stop=False)
                    nc.tensor.matmul(out=ps, lhsT=b1, rhs=ones_row,
                                     start=False, stop=True)
                # v pair-packed: h=0 -> rows 0:64, h=1 -> rows 64:128
                vps = vps_pool.tile([128, 512], FP32)
                for h in (0, 1):
                    vrows = vps[64 * h:64 * h + 64, :]
                    nc.tensor.matmul(out=vrows, lhsT=w2[0],
                                     rhs=xc[0][:, 512 * h:512 * h + 512],
                                     start=True, stop=False)
                    nc.tensor.matmul(out=vrows, lhsT=w2[1],
                                     rhs=xc[1][:, 512 * h:512 * h + 512],
                                     start=False, stop=False)
                    nc.tensor.matmul(out=vrows, lhsT=b2, rhs=ones_row,
                                     start=False, stop=True)

                # ---------- squares (ACT, bf16 out) ----------
                sq = [sq_pool.tile([128, 512], BF16_DT, name=f"sq{h}", tag=f"sq{h}")
                      for h in (0, 1)]
                for h in (0, 1):
                    nc.scalar.square(out=sq[h], in_=ps_qk[h])
                sq_v = sq_pool.tile([128, 512], BF16_DT, name="sqv", tag="sqv")
                nc.scalar.square(out=sq_v, in_=vps)

                # ---------- replicated variance (PE fat-lhsT) ----------
                # One 3-bank PSUM tile: cols 0:512 h0-var, 512:1024 h1, 1024: v
                # (fat lhsT replicates each group's variance down its rows).
                var = var_pool.tile([128, 1536], FP32)
                nc.tensor.matmul(out=var[:, 0:512], lhsT=obf[0], rhs=sq[0],
                                 start=True, stop=True)
                nc.tensor.matmul(out=var[:, 512:1024], lhsT=obf[0], rhs=sq[1],
                                 start=True, stop=True)
                nc.tensor.matmul(out=var[:, 1024:1536], lhsT=obf[1], rhs=sq_v,
                                 start=True, stop=True)

                # ---------- rstd (ONE raw Rsqrt pass for q,k,v) ----------
                rstd = rstd_pool.tile([128, 1536], FP32)
                _act_rsqrt(nc, rstd, var, eps_sb)
                reps = [rstd[:, 0:512], rstd[:, 512:1024], rstd[:, 1024:1536]]

                # ---------- normalize + relu (DVE) ----------
                qkn = []
                for h in (0, 1):
                    z = z_pool.tile([128, 512], BF16_DT, name=f"z{h}", tag=f"z{h}")
                    nc.vector.tensor_mul(out=z, in0=ps_qk[h], in1=reps[h])
                    qn = qn_pool.tile([128, 512], BF16_DT, name=f"qkn{h}", tag=f"qkn{h}")
                    qkn.append(qn)
                    nc.vector.tensor_scalar(
                        out=qn, in0=z, scalar1=betas[:, 0:1], scalar2=0.0,
                        op0=mybir.AluOpType.add, op1=mybir.AluOpType.max)
                z_v = z_pool.tile([128, 512], BF16_DT, name="zv", tag="zv")
                nc.vector.tensor_mul(out=z_v, in0=vps, in1=reps[2])
                vn = qn_pool.tile([128, 512], BF16_DT, name="vn", tag="vn")
                nc.vector.tensor_scalar(
                    out=vn, in0=z_v, scalar1=betas[:, 1:2], scalar2=0.0,
                    op0=mybir.AluOpType.add, op1=mybir.AluOpType.max)

                # ---------- batched token-major transposes (xbar DMA) ----------
                # k rows: h0 -> qkn[0][64:128], h1 -> qkn[1][0:64] (swapped)
                ktm = [tm_pool.tile([128, 4, 64], BF16_DT, name=f"ktm{h}", tag=f"ktm{h}")
                       for h in (0, 1)]
                nc.sync.dma_start_transpose(out=ktm[0], in_=qkn[0][64:128, :])
                nc.sync.dma_start_transpose(out=ktm[1], in_=qkn[1][64:128, :])
                vtm = tm_pool.tile([128, 4, 128], BF16_DT, name="vtm", tag="vtm")
                nc.sync.dma_start_transpose(out=vtm, in_=vn[:, :])

                # ---------- kT @ v (four 64x64 Ms in one bank, col-packed) ---
                mps = mps_pool.tile([64, 256], FP32)
                for h in (0, 1):
                    for ss in (0, 1):
                        mcol = 64 * (2 * h + ss)
                        for cc in (0, 1):
                            ch = 2 * ss + cc
                            nc.tensor.matmul(
                                out=mps[:, mcol:mcol + 64],
                                lhsT=ktm[h][:, ch, :],
                                rhs=vtm[:, ch, 64 * h:64 * h + 64],
                                start=(cc == 0), stop=(cc == 1))
                msb = msb_pool.tile([64, 256], BF16_DT)
                nc.vector.tensor_copy(out=msb, in_=mps)

                # ---------- att = q @ M ----------
                # q rows: h0 -> 0:64, h1 -> 64:128 (swapped layout); the two h
                # streams use disjoint PE row strips and run concurrently.
                atps = atps_pool.tile([128, 512], FP32)
                for h in (0, 1):
                    for ss in (0, 1):
                        mcol = 64 * (2 * h + ss)
                        for bt in (0, 1):
                            col = 256 * bt + 128 * h + 64 * ss
                            nc.tensor.matmul(
                                out=atps[:, col:col + 64],
                                lhsT=qkn[h][0:64,
                                            256 * ss + 128 * bt:
                                            256 * ss + 128 * bt + 128],
                                rhs=msb[:, mcol:mcol + 64],
                                start=True, stop=True)
                atts = ao_pool.tile([128, 512], FP32)
                nc.vector.tensor_copy(out=atts, in_=atps)

                # ---------- output ----------
                # atts col = 256bt + 128h + 64ss + j ; s = 4p + 2h + ss
                for bt in (0, 1):
                    nc.sync.dma_start(
                        out=out[128 * bt:128 * bt + 128,
                                4 * p:4 * p + 4, :].rearrange(
                            "pp (h ss) j -> pp h ss j", h=2, ss=2),
                        in_=atts[:, 256 * bt:256 * bt + 256].rearrange(
                            "pp (h ss j) -> pp h ss j", h=2, ss=2, j=64))
    _legalize_waits(nc)
    return nc


def prepare_host_inputs(inputs):
    """Precompute the shared (weight-derived) device inputs."""
    def fold(W, b, g):
        Wc = W - W.mean(axis=1, keepdims=True)
        bc = b - b.mean()
        return (Wc * g[None, :]), (bc * g)

    Wq, bq = fold(inputs["Wq"], inputs["bq"], inputs["gq"])
    Wk, bk = fold(inputs["Wk"], inputs["bk"], inputs["gk"])
    Wv, bv = fold(inputs["Wv"], inputs["bv"], inputs["gv"])

    w1 = np.concatenate([Wq, Wk], axis=1).reshape(2, 128, 128)
    b1 = np.concatenate([bq, bk]).reshape(1, 128)
    w2 = Wv.reshape(2, 128, 64)
    b2 = bv.reshape(1, 64)

    inv2q = (1.0 / np.square(inputs["gq"])) / D
    inv2k = (1.0 / np.square(inputs["gk"])) / D
    inv2v = (1.0 / np.square(inputs["gv"])) / D
    # fat stat lhsT: col p gets the group weights of its rows (replicates var)
    obf = np.zeros((2, 128, 128), np.float32)
    obf[0, 0:64, 0:64] = inv2q[:, None]
    obf[0, 64:128, 64:128] = inv2k[:, None]
    obf[1, 0:64, 0:64] = inv2v[:, None]
    obf[1, 64:128, 64:128] = inv2v[:, None]

    betas = np.zeros((128, 2), np.float32)
    betas[0:64, 0] = inputs["betaq"]
    betas[64:128, 0] = inputs["betak"]
    betas[0:64, 1] = inputs["betav"]
    betas[64:128, 1] = inputs["betav"]

    return {
        "w1d": np.ascontiguousarray(w1.astype(BF16)),
        "b1d": np.ascontiguousarray(b1.astype(BF16)),
        "w2d": np.ascontiguousarray(w2.astype(BF16)),
        "b2d": np.ascontiguousarray(b2.astype(BF16)),
        "obfd": np.ascontiguousarray(obf.astype(BF16)),
        "betad": betas,
    }


def make_xT(x_bf16, s0, s1):
    """x[b, s0:s1, f] -> xT[chunk, p, s_local*B + b] (bf16)."""
    xc = x_bf16[:, s0:s1, :]                      # [B, SLc, F]
    xt = np.transpose(xc, (2, 1, 0))              # [F, SLc, B]
    return np.ascontiguousarray(xt.reshape(2, 128, -1))


_cached = {}
_last_results = {}


def kernel(**inputs):
    x = np.asarray(inputs["inputs"], np.float32)
    Bx, Sx, Fx = x.shape
    assert (Bx, Sx, Fx) == (B, S, F)

    T = SL * B
    if "nc" not in _cached:
        _cached["nc"] = build_kernel(T)
    nc = _cached["nc"]

    shared = prepare_host_inputs({k: np.asarray(v, np.float32)
                                  for k, v in inputs.items() if k != "inputs"})
    x_bf16 = x.astype(BF16)
    in_maps = []
    for core in range(NCORES):
        m = dict(shared)
        m["xT"] = make_xT(x_bf16, core * SL, (core + 1) * SL)
        in_maps.append(m)

    trace = bool(os.environ.get("BASS_TRACE"))
    res = run_bass_kernel_spmd(nc, in_maps, core_ids=list(range(NCORES)),
                               trace=trace)
    _last_results["res"] = res
    # per-core output is [B, SL, D]; concatenate along s
    return np.concatenate([r["att"] for r in res.results], axis=1)


if __name__ == "__main__":
    rng = np.random.default_rng(0)
    fake = {
        "inputs": rng.standard_normal((B, S, F), dtype=np.float32),
        "Wq": rng.standard_normal((F, D), dtype=np.float32) / 16,
        "bq": rng.standard_normal(D).astype(np.float32) * 0.01,
        "gq": 1 + rng.standard_normal(D).astype(np.float32) * 0.01,
        "Wk": rng.standard_normal((F, D), dtype=np.float32) / 16,
        "bk": rng.standard_normal(D).astype(np.float32) * 0.01,
        "gk": 1 + rng.standard_normal(D).astype(np.float32) * 0.01,
        "Wv": rng.standard_normal((F, D), dtype=np.float32) / 16,
        "bv": rng.standard_normal(D).astype(np.float32) * 0.01,
        "gv": 1 + rng.standard_normal(D).astype(np.float32) * 0.01,
        "betaq": rng.standard_normal(D).astype(np.float32) * 0.01,
        "betak": rng.standard_normal(D).astype(np.float32) * 0.01,
        "betav": rng.standard_normal(D).astype(np.float32) * 0.01,
    }
    out = kernel(**fake)
    print("kernel output", out.shape, out.dtype, float(np.abs(out).max()))
